# revision 15
# baseline (speedup 1.0000x reference)
"""Trainium2 Bass kernel for nn_CentroidModel (retrieval_knn).

Computes out = -(||e||^2 + ||c||^2 - 2 e.c) with e = x @ W, over 8
NeuronCores, data-parallel on the batch dim (x rows sharded; W and
centroids replicated).

All GEMMs run in fp8e4 (e4m3) with MatmulPerfMode.DoubleRow: operands
are packed [128, 2, free] so each matmul contracts 256 rows — ~1.5x PE
throughput over bf16 at free-dim 512.  The error budget (rel 2e-2 of an
output whose magnitude is dominated by ||e||^2+||c||^2 ~ 1100) is ~28
absolute; fp8 quantization of e and c contributes ~3, fp16 output
rounding ~1.

Math per core (b_loc = B/8 batch rows):
    et2  = fp8(2 * (x @ W).T)       (fp16 matmul — same PE rate as bf16
                                     but 10-bit mantissa, so the only
                                     e-side quantization is the single
                                     fp8 cast of et2; ACT evacuates
                                     psum with scale 2)
    e_sq = ONES.T @ (et2*et2) / 4   (DVE square + ONES-matmul partition
                                     reduction + PE transpose, scale -1/4)
    c_sq = ONES.T @ sum_k(ct*ct)    (DVE square+add, ONES-matmul, ACT
                                     copy to fp16)
    out  = fp16((et2.T @ ct) - e_sq - c_sq)
           = DoubleRow matmul + ACT bias(-e_sq) + DVE sub(c_sq)

Inputs arrive pre-packed from the host in DoubleRow pair layout
[(j,p), k2, free] where contraction index = (2j + k2)*128 + p; outputs
are stored fp16 and upcast to f32 on the host.
"""

import numpy as np

_B, _DIN, _D, _C = 8192, 1024, 768, 16384
_NCORES = 8
_B_LOC = _B // _NCORES

_P = 128  # SBUF/PSUM partitions
_NT = 512  # matmul free-dim tile (one f32 PSUM bank)
_NW = 1024  # main-loop working width (two-bank PSUM tiles)

_JX = _DIN // (2 * _P)  # k-pairs over d_in (4)
_JD = _D // (2 * _P)  # k-pairs over d (3)


def emit_centroid_kernel(tc, xt, w, ct, out, b_loc, din, d, c):
    """Emit the per-core Tile kernel.

    xt:  [din, b_loc] fp16           (x shard, pre-transposed)
    w:   [din, d] fp16
    ct:  [(jd,p), 2, c] fp8e4        (centroids.T, DoubleRow-packed)
    out: [b_loc, c] fp16
    """
    from concourse import mybir
    from concourse.masks import make_identity

    nc = tc.nc
    e4 = mybir.dt.float8e4
    bf16 = mybir.dt.bfloat16
    f16 = mybir.dt.float16
    f32 = mybir.dt.float32
    AF = mybir.ActivationFunctionType
    DR = mybir.MatmulPerfMode.DoubleRow

    kd = din // _P  # k-tiles over d_in
    jd = d // (2 * _P)  # k-pairs over d (embedding)
    md = d // _P  # 128-blocks over d
    mb = b_loc // _P  # tiles over local batch
    npair = c // _NW  # c-chunks

    with (
        tc.tile_pool(name="persist", bufs=1) as persist,
        tc.tile_pool(name="ct_in", bufs=9) as ct_pool,
        tc.tile_pool(name="sq", bufs=8) as sq_pool,
        tc.tile_pool(name="csqs", bufs=3) as csq_pool,
        tc.tile_pool(name="t1", bufs=6) as t1_pool,
        tc.tile_pool(name="outs", bufs=10) as out_pool,
        tc.tile_pool(name="scratch", bufs=2) as scratch,
    ):
        # ---- persistent SBUF tensors ----
        xt_s = [persist.tile([_P, b_loc], f16, name=f"xt{k}", tag=f"xt{k}") for k in range(kd)]
        w_s = [persist.tile([_P, d], f16, name=f"w{k}", tag=f"w{k}") for k in range(kd)]
        et2_s = [persist.tile([_P, 2, b_loc], e4, name=f"et{j}", tag=f"et{j}") for j in range(jd)]
        negesq = persist.tile([_P, mb], f32, name="negesq", tag="negesq")
        ones = persist.tile([_P, _P], bf16, name="ones", tag="ones")
        ident = persist.tile([_P, _P], f32, name="ident", tag="ident")

        for k in range(kd):
            nc.sync.dma_start(xt_s[k][:], xt[k * _P : (k + 1) * _P, :])
            nc.sync.dma_start(w_s[k][:], w[k * _P : (k + 1) * _P, :])
        # emitted after the input loads: only needed by e_sq/c_sq, and
        # emitting them first would delay the PE-critical xt/w arrival
        nc.vector.memset(ones[:], 1.0)
        make_identity(nc, ident[:])

        # ---- prologue (own PSUM scope, freed before the main loop) ----
        with tc.tile_pool(name="ps_pro", bufs=2, space="PSUM") as ps_pro:
            # phase 1: et2 = fp8(2 * (W.T @ xT)) in fp16
            for m in range(md):
                pt = ps_pro.tile([_P, b_loc], f32, name="pro", tag="pro")
                for nb in range(b_loc // _NT):
                    bs = slice(nb * _NT, (nb + 1) * _NT)
                    for k in range(kd):
                        nc.tensor.matmul(
                            pt[:, bs],
                            w_s[k][:, m * _P : (m + 1) * _P],
                            xt_s[k][:, bs],
                            start=(k == 0),
                            stop=(k == kd - 1),
                        )
                nc.scalar.activation(
                    et2_s[m // 2][:, m % 2, :],
                    pt[:],
                    AF.Copy,
                    scale=2.0,
                )

            # phase 2: e_sq = sum_d e^2  (= partition-reduce of (et2)^2 / 4)
            pesq = ps_pro.tile([_P, b_loc], f32, name="pesq", tag="pesq", bufs=1)
            nmm = 2 * jd
            for j in range(jd):
                sqe = scratch.tile([_P, 2, b_loc], bf16, name="sqe", tag="sqe")
                nc.vector.tensor_mul(sqe[:], et2_s[j][:], et2_s[j][:])
                for k2 in range(2):
                    for nb in range(b_loc // _NT):
                        bs = slice(nb * _NT, (nb + 1) * _NT)
                        nc.tensor.matmul(
                            pesq[:, bs],
                            ones[:],
                            sqe[:, k2, bs],
                            start=(j == 0 and k2 == 0),
                            stop=(j == jd - 1 and k2 == 1),
                        )
            # every row of pesq holds 4*e_sq[b]; PE-transpose each
            # [128, 128] slice so e_sq lands per-partition, scale by -1/4.
            esq_rep = scratch.tile([_P, b_loc], f32, name="esq_rep", tag="esq_rep")
            nc.scalar.activation(esq_rep[:], pesq[:], AF.Copy)
            for i in range(mb):
                ptr = ps_pro.tile([_P, _P], f32, name="ptr", tag="ptr")
                nc.tensor.transpose(ptr[:], esq_rep[:, i * _P : (i + 1) * _P], ident[:])
                nc.scalar.activation(negesq[:, i : i + 1], ptr[:, 0:1], AF.Copy, scale=-0.25)

        # ---- main loop over c-chunks (1024 centroids each) ----
        with (
            tc.tile_pool(name="ps_big", bufs=3, space="PSUM") as ps_big,
            tc.tile_pool(name="ps_csq", bufs=1, space="PSUM") as ps_csq,
        ):
            def load_ct(n):
                csl = slice(n * _NW, (n + 1) * _NW)
                tiles = []
                for j in range(jd):
                    t = ct_pool.tile([_P, 2, _NW], e4, name=f"ct{j}", tag="ct")
                    nc.sync.dma_start(t[:], ct[j * _P : (j + 1) * _P, :, csl])
                    tiles.append(t)
                return tiles

            def emit_csq(ct_t):
                # sum_k ct^2 via DVE square + adds (all-DVE: GpSimd shares
                # SBUF ports with DVE and would slow it ~4x), then one
                # 1024-wide ONES-matmul, evacuated to fp16 SBUF by ACT.
                sqs = []
                for j in range(jd):
                    sq_t = sq_pool.tile([_P, 2, _NW], bf16, name="sqc", tag="sqc")
                    nc.vector.tensor_mul(sq_t[:], ct_t[j][:], ct_t[j][:])
                    sqs.append(sq_t)
                nc.vector.tensor_add(sqs[0][:], sqs[0][:], sqs[1][:])
                nc.vector.tensor_add(sqs[0][:], sqs[0][:], sqs[2][:])
                csum = sq_pool.tile([_P, _NW], bf16, name="csum", tag="csum")
                nc.vector.tensor_add(csum[:], sqs[0][:, 0, :], sqs[0][:, 1, :])
                pcs = ps_csq.tile([_P, _NW], f32, name="csq", tag="csq")
                for h in range(2):
                    hs = slice(h * _NT, (h + 1) * _NT)
                    nc.tensor.matmul(pcs[:, hs], ones[:], csum[:, hs], start=True, stop=True)
                csq_s = csq_pool.tile([_P, _NW], f16, name="csq_s", tag="csq_s")
                nc.scalar.activation(csq_s[:], pcs[:], AF.Copy)
                return csq_s

            # csq is computed one chunk AHEAD of its consumers so the PE
            # never head-of-line stalls on the DVE square/add chain; output
            # stores are flushed one chunk LATE and issued on the ACT HWDGE
            # ring so ct loads never queue behind them on the SP ring.
            pending_stores = []
            ct_cur = load_ct(0)
            csq_cur = emit_csq(ct_cur)
            for n in range(npair):
                csl = slice(n * _NW, (n + 1) * _NW)
                ct_nxt = load_ct(n + 1) if n + 1 < npair else None
                for dst, src_t in pending_stores:
                    nc.scalar.dma_start(dst, src_t[:])
                pending_stores = []

                for i in range(mb):
                    # j-outer / half-inner: consecutive matmuls share the
                    # stationary operand, so the PE reuses loaded weights
                    pb = ps_big.tile([_P, _NW], f32, name="big", tag="big")
                    for j in range(jd):
                        lhsT = et2_s[j][:, :, i * _P : (i + 1) * _P]
                        nc.tensor.matmul(
                            pb[:, 0:_NT], lhsT, ct_cur[j][:, :, 0:_NT],
                            start=(j == 0), stop=(j == jd - 1), perf_mode=DR,
                        )
                        nc.tensor.matmul(
                            pb[:, _NT:_NW], lhsT, ct_cur[j][:, :, _NT:_NW],
                            start=(j == 0), stop=(j == jd - 1), perf_mode=DR,
                        )
                    t1 = t1_pool.tile([_P, _NW], f16, name="t1", tag="t1")
                    nc.scalar.activation(
                        t1[:], pb[:], AF.Identity, bias=negesq[:, i : i + 1]
                    )
                    ot = out_pool.tile([_P, _NW], f16, name="ot", tag="ot")
                    nc.vector.tensor_sub(ot[:], t1[:], csq_cur[:])
                    if n == npair - 1:
                        nc.scalar.dma_start(out[i * _P : (i + 1) * _P, csl], ot[:])
                    else:
                        pending_stores.append((out[i * _P : (i + 1) * _P, csl], ot))
                    if i == 0 and ct_nxt is not None:
                        csq_nxt = emit_csq(ct_nxt)
                if ct_nxt is not None:
                    ct_cur, csq_cur = ct_nxt, csq_nxt
            for dst, src_t in pending_stores:
                nc.scalar.dma_start(dst, src_t[:])


def build_nc(b_loc=_B_LOC, din=_DIN, d=_D, c=_C):
    import concourse.tile as tile
    from concourse import bacc, mybir

    nc = bacc.Bacc("TRN2", target_bir_lowering=False, debug=False)
    jd = d // (2 * _P)
    xt = nc.declare_dram_parameter("xt", [din, b_loc], mybir.dt.float16, isOutput=False)
    w = nc.declare_dram_parameter("w", [din, d], mybir.dt.float16, isOutput=False)
    ct = nc.declare_dram_parameter("ct", [jd * _P, 2, c], mybir.dt.float8e4, isOutput=False)
    out = nc.declare_dram_parameter("out", [b_loc, c], mybir.dt.float16, isOutput=True)
    with tile.TileContext(nc) as tc:
        emit_centroid_kernel(tc, xt.ap(), w.ap(), ct.ap(), out.ap(), b_loc, din, d, c)
    nc.compile()
    return nc


def _pack_pairs(a2d, dtype):
    """[K, F] -> [(j,p), 2, F] DoubleRow pair layout, row (2j+k2)*128+p."""
    k, f = a2d.shape
    j = k // (2 * _P)
    return np.ascontiguousarray(
        a2d.reshape(j, 2, _P, f).transpose(0, 2, 1, 3).reshape(j * _P, 2, f)
    ).astype(dtype)


def make_in_maps(x, W, centroids, b_loc=_B_LOC, n_cores=_NCORES):
    import ml_dtypes

    e4 = ml_dtypes.float8_e4m3

    x = np.asarray(x, dtype=np.float32)
    W = np.asarray(W, dtype=np.float32)
    centroids = np.asarray(centroids, dtype=np.float32)

    w_f16 = W.astype(np.float16)  # [DIN, D]
    ct_p = _pack_pairs(np.ascontiguousarray(centroids.T), e4)  # [(jd,p), 2, C]
    xt_full = np.ascontiguousarray(x.T).astype(np.float16)  # [DIN, B]

    maps = []
    for i in range(n_cores):
        xt_p = np.ascontiguousarray(xt_full[:, i * b_loc : (i + 1) * b_loc])
        maps.append({"xt": xt_p, "w": w_f16, "ct": ct_p})
    return maps


_NC_CACHE = {}


def kernel(x, W, centroids):
    from concourse.bass_utils import run_bass_kernel_spmd

    if "nc" not in _NC_CACHE:
        _NC_CACHE["nc"] = build_nc()
    nc = _NC_CACHE["nc"]

    in_maps = make_in_maps(x, W, centroids)
    res = run_bass_kernel_spmd(nc, in_maps, list(range(_NCORES)))
    return np.concatenate(
        [res.results[i]["out"].astype(np.float32) for i in range(_NCORES)], axis=0
    )
